# revision 4
# baseline (speedup 1.0000x reference)
"""Trainium2 Bass kernel for nn_LocalInteractionsLayer.

Reference computation:
    seq_pairs [B=16, C=8, L=4096, 2] f32
    top = seq_pairs[..., 0]; bot = seq_pairs[..., 1]
    out[b, p, c*225 + i*15 + j] = top[b, c, p+i] * bot[b, c, p+j]
    for p in [0, P), i,j in [0, 15), P = L - 14 = 4082
    -> out [16, 4082, 1800] f32 (~470 MB; heavily output-write bound).

Strategy:
  - Data-parallel over batch: 2 batches per core on 8 cores.
  - Host pre-builds the 15-wide sliding windows (a 15x data expansion of the
    tiny 4 MB input) laid out so each SBUF partition p holds the windows for
    output position t*128+p contiguously. One fully-contiguous DMA load per
    8-tile group brings in both top and bot windows.
  - On device, one vector-engine tensor_mul per (128-position tile, channel)
    computes a [128, 15, 15] outer-product block using broadcast (step-0)
    access patterns (S3S3D3_TT: 3-D APs). Output tile [128, 1800] is stored
    with one fully-contiguous ~921 KB DMA per tile.
"""

import sys

if "/opt/trn_rl_repo" not in sys.path:
    sys.path.insert(0, "/opt/trn_rl_repo")

import numpy as np
from numpy.lib.stride_tricks import sliding_window_view

import concourse.tile as tile
from concourse import bacc, mybir
from concourse.bass_utils import run_bass_kernel_spmd

W = 15            # window length (2*7+1)
WPAD = W - 1
B, C, L = 16, 8, 4096
P = L - WPAD      # 4082 valid output positions
FREE = C * W * W  # 1800
NCORES = 8
BPC = B // NCORES  # batches per core = 2
NT = L // 128      # 32 position-tiles per batch (last one partially valid)
NG = 4             # tile groups per batch (DMA load batching)
GT = NT // NG      # 8 tiles per group
GW = GT * C * W    # free size of one operand group = 960

_BUILD_CACHE: dict = {}


def _build(loop_iters: int = 1):
    """Build + compile the per-core Bacc program (identical on all 8 cores)."""
    nc = bacc.Bacc("TRN2", target_bir_lowering=False, debug=False, num_devices=NCORES)
    dt = mybir.dt.float32

    # inw[b, g, :, 0:GW] = top windows, [.., GW:2*GW] = bot windows
    inw_d = nc.dram_tensor("inw", [BPC, NG, 128, 2 * GW], dt, kind="ExternalInput")
    out_d = nc.dram_tensor("out", [BPC, P, FREE], dt, kind="ExternalOutput")

    with tile.TileContext(nc) as tc:
        with (
            tc.tile_pool(name="inp", bufs=3) as inp,
            tc.tile_pool(name="outp", bufs=4) as outp,
        ):
            def _body(_it=None):
                for b in range(BPC):
                    for g in range(NG):
                        inwt = inp.tile([128, 2 * GW], dt, tag="inw")
                        nc.sync.dma_start(inwt[:], inw_d[b, g])
                        for tq in range(GT):
                            t = g * GT + tq
                            ot = outp.tile([128, FREE], dt, tag="ot")
                            for c in range(C):
                                a = (
                                    inwt[:, tq * C * W + c * W : tq * C * W + (c + 1) * W]
                                    .unsqueeze(2)
                                    .broadcast_to((128, W, W))
                                )
                                bb = (
                                    inwt[:, GW + tq * C * W + c * W : GW + tq * C * W + (c + 1) * W]
                                    .unsqueeze(1)
                                    .broadcast_to((128, W, W))
                                )
                                o = ot[:, c * W * W : (c + 1) * W * W].rearrange(
                                    "p (i j) -> p i j", i=W
                                )
                                nc.vector.tensor_mul(o, a, bb)
                            rows = min(128, P - t * 128)
                            nc.sync.dma_start(
                                out_d[b, t * 128 : t * 128 + rows, :], ot[:rows, :]
                            )

            if loop_iters == 1:
                _body()
            else:
                with tc.For_i(0, loop_iters, 1) as it:
                    _body(it)
    nc.compile()
    return nc


def _get_built(loop_iters: int = 1):
    nc = _BUILD_CACHE.get(loop_iters)
    if nc is None:
        nc = _build(loop_iters)
        _BUILD_CACHE[loop_iters] = nc
    return nc


def _prep(seq_pairs: np.ndarray) -> np.ndarray:
    """Host-side window expansion into the DMA-friendly device layout.

    inw[b, g, p, s*GW + tq*C*W + c*W + i] = seq_pairs[b, c, (g*GT+tq)*128 + p + i, s]
    (positions past P-1 read zero padding; those rows are never stored).
    """
    sp = np.ascontiguousarray(seq_pairs, dtype=np.float32)
    padded = np.zeros((B, C, L + WPAD, 2), np.float32)
    padded[:, :, :L] = sp
    win = sliding_window_view(padded, W, axis=2)  # [B, C, L, 2, W]
    v = win.reshape(B, C, NG, GT, 128, 2, W)
    v = np.ascontiguousarray(v.transpose(0, 2, 4, 5, 3, 1, 6))  # [b,g,p,s,tq,c,i]
    return v.reshape(B, NG, 128, 2 * GW)


def kernel(seq_pairs: np.ndarray) -> np.ndarray:
    inw = _prep(seq_pairs)
    nc = _get_built()
    in_maps = [{"inw": inw[k * BPC : (k + 1) * BPC]} for k in range(NCORES)]
    res = run_bass_kernel_spmd(nc, in_maps, list(range(NCORES))).results
    return np.concatenate([res[k]["out"] for k in range(NCORES)], axis=0)
